# revision 24
# baseline (speedup 1.0000x reference)
"""Causal multi-head attention (B=2, N=2048, D=1024, H=16, Dh=64) on 8 trn2 cores.

Sharding: core c = (batch b = c//4, head-quadrant g = c%4) -> 4 heads of one
batch per core.  bf16 datapath (fp32 PSUM accumulation):
  - Q^T/K^T per head-pair projected from host-pretransposed x^T (bf16),
  - causal flash-style attention in the transposed orientation
    (s_T[j,q] = K^T.T @ Q^T, exp on ScalarE -> bf16, causal mask applied
    post-exp multiplicatively on DVE, P.V + row-sum matmuls in PSUM),
  - software-pipelined emission: PV lags QK by one batch, normalize lags
    its strip by one strip, diagonal (masked) batches run first in each
    strip, pair-1 QK projection and the output projection gap-fill the PE
    between strips,
  - per-strip normalize 1/s = exp(-ln s) on ScalarE (one fused [1,1024]
    row for both heads), partition broadcast on GpSimd, multiply on DVE.
Host sums the 4 partial output projections per batch.
"""

import numpy as np
import ml_dtypes

B, N, D, H, Dh = 2, 2048, 1024, 16, 64
DC = D // 128          # 8 contraction chunks
NB = N // 128          # 16 ctx blocks
NS = N // 512          # 4 q strips
N_CORES = 8
SCALE = float(Dh) ** -0.5

_COMPILED = None
TRACE = False
LAST_EXEC_NS = None
LAST_RESULTS = None


def _build():
    import concourse.bass as bass
    import concourse.tile as tile
    from concourse import bacc, mybir

    f32 = mybir.dt.float32
    bf = mybir.dt.bfloat16
    EXP = mybir.ActivationFunctionType.Exp
    LN = mybir.ActivationFunctionType.Ln

    nc = bacc.Bacc("TRN2", target_bir_lowering=False, debug=False,
                   enable_asserts=False, num_devices=N_CORES)

    # Both Exp and Ln are used (softmax exp + exp(-ln s) normalize).  The
    # table picker binds each function to the first set containing it,
    # which splits them across two sets and inserts a ~1.3us
    # ACT_TABLE_LOAD per switch (17 loads).  Strip Exp/Ln from every other
    # set (in the cached dict, order/ids preserved) so both bind to
    # natural_log_exp_and_others -> a single load.
    import concourse.hw_specs as hw_specs
    tabs = hw_specs.get_activation_tables(nc.m.arch)
    for name, funcs in tabs.items():
        if name != "natural_log_exp_and_others":
            funcs.discard(EXP)
            funcs.discard(LN)

    xT = nc.dram_tensor("xT", [D, N], bf, kind="ExternalInput")
    # weights are host-preswizzled to the SBUF layout [128, c, n] so the
    # DMAs are contiguous 4KB-per-partition (strided 512B-segment loads get
    # starved to ~1/15th bandwidth behind the x^T chunk stream)
    wq = nc.dram_tensor("wq", [128, DC * 256], bf, kind="ExternalInput")
    wk = nc.dram_tensor("wk", [128, DC * 256], bf, kind="ExternalInput")
    wv = nc.dram_tensor("wv", [128, DC * 256], bf, kind="ExternalInput")
    wo = nc.dram_tensor("wo", [128, 2 * D], bf, kind="ExternalInput")
    bo = nc.dram_tensor("bo", [D], f32, kind="ExternalInput")
    kmask = nc.dram_tensor("kmask", [128, 128], bf, kind="ExternalInput")
    y = nc.dram_tensor("y", [N, D], f32, kind="ExternalOutput")

    with tile.TileContext(nc) as tc:
        from contextlib import ExitStack
        with ExitStack() as ctx:
            const = ctx.enter_context(tc.tile_pool(name="const", bufs=1))
            work = ctx.enter_context(tc.tile_pool(name="work", bufs=3))
            epool = ctx.enter_context(tc.tile_pool(name="epool", bufs=5))
            pssT = ctx.enter_context(
                tc.tile_pool(name="pssT", bufs=2, space=bass.MemorySpace.PSUM))
            pspv = ctx.enter_context(
                tc.tile_pool(name="pspv", bufs=2, space=bass.MemorySpace.PSUM))

            # ---------------- loads ----------------
            # ONE serial Sync queue in exact consumption order: parallel
            # queues round-robin packets, which spreads every transfer's
            # completion across the whole ~18us load window; serial FIFO
            # gives wq at ~4us and one x^T chunk every ~1.6us, matching the
            # d-interleaved projection's consumption rate.
            wq_sb = const.tile([128, DC, 256], bf)
            nc.sync.dma_start(wq_sb[:].rearrange("p c n -> p (c n)"), wq.ap())
            wk_sb = const.tile([128, DC, 256], bf)
            nc.sync.dma_start(wk_sb[:].rearrange("p c n -> p (c n)"), wk.ap())

            xs = []
            for d in range(DC):
                xd = const.tile([128, N], bf, name=f"x{d}")
                nc.sync.dma_start(xd[:], xT.ap()[128 * d:128 * d + 128, :])
                xs.append(xd)

            wv_sb = const.tile([128, DC, 256], bf)
            nc.sync.dma_start(wv_sb[:].rearrange("p c n -> p (c n)"), wv.ap())
            wo_sb = const.tile([128, 2, D], bf)
            nc.sync.dma_start(wo_sb[:].rearrange("p c n -> p (c n)"), wo.ap())
            kmask_sb = const.tile([128, 128], bf)
            nc.sync.dma_start(kmask_sb[:], kmask.ap())
            bo_sb = const.tile([1, D], f32)
            nc.sync.dma_start(bo_sb[:], bo.ap().rearrange("(a n) -> a n", a=1))
            Bb = const.tile([128, D], f32)
            nc.gpsimd.partition_broadcast(Bb[:], bo_sb[0:1, :])

            # ---------------- QKV projections ----------------
            qkdst = {}

            def proj_qk_part(p, w_sb, nm, half, interleave=False):
                """One [128,1024] accumulation group of the Q^T or K^T
                projection for head-pair p (emitted piecewise so the groups
                can be spread across the schedule)."""
                key = (nm, p)
                if key not in qkdst:
                    qkdst[key] = const.tile([128, N], bf, tag=f"{nm}T{p}",
                                            name=f"{nm}T{p}")
                dst = qkdst[key]
                hsl = slice(1024 * half, 1024 * half + 1024)
                pq = pssT.tile([128, 1024], f32, tag="sT",
                               name=f"p{nm}{p}{half}")
                for d in range(DC):
                    for ns in (0, 1):
                        osl = slice(512 * ns, 512 * ns + 512)
                        nsl = slice(1024 * half + 512 * ns,
                                    1024 * half + 512 * ns + 512)
                        nc.tensor.matmul(
                            pq[:, osl], w_sb[:, d, 128 * p:128 * p + 128],
                            xs[d][:, nsl],
                            start=(d == 0), stop=(d == DC - 1),
                            skip_group_check=interleave)
                nc.vector.tensor_copy(dst[:, hsl], pq[:])
                return dst

            def proj_qk0():
                # pair 0, Q and K interleaved per d-chunk so the PE consumes
                # the x^T DMA stream as it lands
                qdst = kdst = None
                for half in (0, 1):
                    hsl = slice(1024 * half, 1024 * half + 1024)
                    pq = pssT.tile([128, 1024], f32, tag="sT", name=f"pq0{half}")
                    pk = pssT.tile([128, 1024], f32, tag="sT", name=f"pk0{half}")
                    for d in range(DC):
                        for ns in (0, 1):
                            osl = slice(512 * ns, 512 * ns + 512)
                            nsl = slice(1024 * half + 512 * ns,
                                        1024 * half + 512 * ns + 512)
                            nc.tensor.matmul(
                                pq[:, osl], wq_sb[:, d, 0:128], xs[d][:, nsl],
                                start=(d == 0), stop=(d == DC - 1),
                                skip_group_check=True)
                            nc.tensor.matmul(
                                pk[:, osl], wk_sb[:, d, 0:128], xs[d][:, nsl],
                                start=(d == 0), stop=(d == DC - 1),
                                skip_group_check=True)
                    if qdst is None:
                        qdst = qkdst[("q", 0)] = const.tile(
                            [128, N], bf, tag="qT0", name="qT0")
                        kdst = qkdst[("k", 0)] = const.tile(
                            [128, N], bf, tag="kT0", name="kT0")
                    nc.vector.tensor_copy(qdst[:, hsl], pq[:])
                    nc.vector.tensor_copy(kdst[:, hsl], pk[:])
                return qdst, kdst

            vsb = const.tile([128, NB, 4, Dh + 1], bf)
            nc.vector.memset(vsb[:, :, :, Dh:Dh + 1], 1.0)

            def proj_v():
                for nb in range(NB):
                    pvp = pspv.tile([128, 256], f32, tag="pv", name=f"pvp{nb}")
                    for d in range(DC):
                        nc.tensor.matmul(
                            pvp[:], xs[d][:, 128 * nb:128 * nb + 128],
                            wv_sb[:, d, :], start=(d == 0), stop=(d == DC - 1))
                    nc.vector.tensor_copy(
                        vsb[:, nb, :, 0:Dh],
                        pvp[:].rearrange("p (h d) -> p h d", h=4))

            onorm = [const.tile([128, N], bf, tag="onorm0", name="onorm0"),
                     const.tile([128, N], bf, tag="onorm1", name="onorm1")]

            def out_proj_strip(s):
                for qb in range(4 * s, 4 * s + 4):
                    qsl = slice(128 * qb, 128 * qb + 128)
                    yp = pssT.tile([128, 1024], f32, tag="sT", name=f"yp{qb}")
                    for nst in (0, 1):
                        osl = slice(512 * nst, 512 * nst + 512)
                        for p in (0, 1):
                            nc.tensor.matmul(yp[:, osl], onorm[p][:, qsl],
                                             wo_sb[:, p, osl],
                                             start=(p == 0), stop=(p == 1))
                    ysb = work.tile([128, D], f32, tag="ysb", name=f"ysb{qb}")
                    nc.vector.tensor_add(ysb[:], yp[:], Bb[:])
                    nc.sync.dma_start(y.ap()[qsl, :], ysb[:])

            QT = [None, None]
            KT = [None, None]
            pvs = {}

            def attn_strip(p, s):
                """QK / exp / mask / PV for strip s of pair p.  PV lags QK by
                one batch; diagonal (masked) batches first."""
                qsl0 = 512 * s
                pv = pspv.tile([65, 1024], f32, tag="pv", name=f"pv{p}{s}")
                pvs[(p, s)] = pv
                batches = [[(4 * s, 512, 0), (4 * s + 1, 384, 512)],
                           [(4 * s + 2, 256, 0), (4 * s + 3, 128, 256)]]
                for j0 in range(0, 4 * s, 2):
                    batches.append([(j0, 512, 0), (j0 + 1, 512, 512)])
                first_j = batches[0][0][0]
                last_j = batches[-1][-1][0]

                def emit_qk(batch, bi):
                    # all head-a matmuls first so exp_a's input is complete
                    # as early as possible (exp_b trails on ScalarE anyway)
                    tot = batch[-1][2] + batch[-1][1]
                    sTa = pssT.tile([128, 1024], f32, tag="sT",
                                    name=f"sTa{p}{s}{bi}")
                    sTb = pssT.tile([128, 1024], f32, tag="sT",
                                    name=f"sTb{p}{s}{bi}")
                    for half, sT in ((0, sTa), (1, sTb)):
                        hp = slice(64 * half, 64 * half + 64)
                        for (j, w, ofs) in batch:
                            off = 512 - w
                            jsl = slice(128 * j, 128 * j + 128)
                            qs = slice(qsl0 + off, qsl0 + 512)
                            nc.tensor.matmul(sT[:, ofs:ofs + w],
                                             KT[p][hp, jsl], QT[p][hp, qs],
                                             start=True, stop=True)
                    return sTa, sTb, tot

                def emit_exp_mask(batch, bi, sTa, sTb, tot):
                    ea = epool.tile([128, 1024], bf, tag="e",
                                    name=f"ea{p}{s}{bi}")
                    eb = epool.tile([128, 1024], bf, tag="e",
                                    name=f"eb{p}{s}{bi}")
                    nc.scalar.activation(ea[:, 0:tot], sTa[:, 0:tot], EXP,
                                         scale=SCALE)
                    nc.scalar.activation(eb[:, 0:tot], sTb[:, 0:tot], EXP,
                                         scale=SCALE)
                    for (j, w, ofs) in batch:
                        if j >= 4 * s:  # diagonal chunk: zero masked probs
                            nc.vector.tensor_mul(ea[:, ofs:ofs + 128],
                                                 ea[:, ofs:ofs + 128],
                                                 kmask_sb[:])
                            nc.vector.tensor_mul(eb[:, ofs:ofs + 128],
                                                 eb[:, ofs:ofs + 128],
                                                 kmask_sb[:])
                    return ea, eb

                def emit_pv(batch, ea, eb):
                    # head-a PVs first: they only need ea, so the PE is not
                    # stalled on exp_b (which ScalarE finishes ~1us later)
                    for half, e in ((0, ea), (1, eb)):
                        for (j, w, ofs) in batch:
                            off = 512 - w
                            nc.tensor.matmul(pv[0:65, 512 * half + off:
                                                512 * half + 512],
                                             vsb[:, j, 2 * p + half, :],
                                             e[:, ofs:ofs + w],
                                             start=(j == first_j),
                                             stop=(j == last_j),
                                             skip_group_check=True)

                # PV(b) is emitted after QK(b+1): the PE always has the next
                # batch's QK queued while ScalarE works through exp(b), so
                # the exp round-trip is off the PE critical path.  exp(b) is
                # emitted before QK(b+1) so the sT slot reuse (bufs=2) only
                # ever waits on already-emitted readers.
                pend = (batches[0], 0, *emit_qk(batches[0], 0))
                for bi in range(1, len(batches)):
                    ea, eb = emit_exp_mask(*pend)
                    pbatch = pend[0]
                    pend = (batches[bi], bi, *emit_qk(batches[bi], bi))
                    emit_pv(pbatch, ea, eb)
                ea, eb = emit_exp_mask(*pend)
                emit_pv(pend[0], ea, eb)

            def normalize(p, s):
                """o = pv[0:64] * (1 / pv[64]) per head; one fused row for
                both heads.  Emitted one strip late so every hop's deps are
                long-ready (no FIFO head-of-line blocking on any engine)."""
                pv = pvs.pop((p, s))
                on = onorm[p]
                lnr = work.tile([1, 1024], f32, tag="lnr", name=f"lnr{p}{s}")
                scr = work.tile([1, 1024], f32, tag="scr", name=f"scr{p}{s}")
                nc.scalar.activation(lnr[:], pv[64:65, :], LN)
                nc.scalar.activation(scr[:], lnr[:], EXP, scale=-1.0)
                R = work.tile([64, 1024], f32, tag="R", name=f"R{p}{s}")
                nc.gpsimd.partition_broadcast(R[:], scr[:])
                qs = slice(512 * s, 512 * s + 512)
                nc.vector.tensor_mul(on[0:64, qs], pv[0:64, 0:512],
                                     R[:, 0:512])
                nc.vector.tensor_mul(on[64:128, qs], pv[0:64, 512:1024],
                                     R[:, 512:1024])

            # ---------------- schedule ----------------
            QT[0], KT[0] = proj_qk0()
            proj_v()
            attn_strip(0, 0)
            proj_qk_part(1, wq_sb, "q", 0)
            attn_strip(0, 1)
            normalize(0, 0)
            proj_qk_part(1, wq_sb, "q", 1)
            attn_strip(0, 2)
            normalize(0, 1)
            proj_qk_part(1, wk_sb, "k", 0)
            attn_strip(0, 3)
            normalize(0, 2)
            proj_qk_part(1, wk_sb, "k", 1)
            QT[1] = qkdst[("q", 1)]
            KT[1] = qkdst[("k", 1)]
            # pair-1 strips longest-first: the kernel tail (last strip ->
            # normalize -> out-proj -> store) then hangs off the SHORT strip
            attn_strip(1, 3)
            normalize(0, 3)
            attn_strip(1, 2)
            normalize(1, 3)
            out_proj_strip(3)
            attn_strip(1, 1)
            normalize(1, 2)
            out_proj_strip(2)
            attn_strip(1, 0)
            normalize(1, 1)
            out_proj_strip(1)
            normalize(1, 0)
            out_proj_strip(0)

    nc.compile()
    return nc


def _get_compiled():
    global _COMPILED
    if _COMPILED is None:
        _COMPILED = _build()
    return _COMPILED


def kernel(x, w_qkv, w_out, b_out):
    global LAST_EXEC_NS, LAST_RESULTS
    from concourse.bass_utils import run_bass_kernel_spmd

    bf16 = ml_dtypes.bfloat16
    x = np.asarray(x, dtype=np.float32)
    w_qkv = np.asarray(w_qkv, dtype=np.float32)
    w_out = np.asarray(w_out, dtype=np.float32)
    b_out = np.asarray(b_out, dtype=np.float32)

    kmask_np = np.triu(np.ones((128, 128), dtype=bf16), 0)

    nc = _get_compiled()
    in_maps = []
    for c in range(N_CORES):
        b, g = divmod(c, 4)
        hs = [4 * g + i for i in range(4)]
        cols = np.concatenate([np.arange(64 * h, 64 * h + 64) for h in hs])
        # swizzle [D, n] -> SBUF layout [128, c, n] flattened (contiguous DMA)
        def swz(w):
            return np.ascontiguousarray(
                w.reshape(-1, 128, w.shape[1]).transpose(1, 0, 2)
                .reshape(128, -1).astype(bf16))

        in_maps.append({
            "xT": np.ascontiguousarray(x[b].T.astype(bf16)),
            "wq": swz(w_qkv[:, cols]),
            "wk": swz(w_qkv[:, D + cols]),
            "wv": swz(w_qkv[:, 2 * D + cols]),
            "wo": swz(w_out[cols, :]),
            "bo": b_out if g == 0 else np.zeros_like(b_out),
            "kmask": kmask_np,
        })
    res = run_bass_kernel_spmd(nc, in_maps, core_ids=list(range(N_CORES)),
                               trace=TRACE)
    LAST_EXEC_NS = res.exec_time_ns
    LAST_RESULTS = res
    ys = [res.results[c]["y"] for c in range(N_CORES)]
    out = np.stack([ys[0] + ys[1] + ys[2] + ys[3],
                    ys[4] + ys[5] + ys[6] + ys[7]])
    return out.astype(np.float32)


# revision 25
# speedup vs baseline: 1.1844x; 1.1844x over previous
"""Causal multi-head attention (B=2, N=2048, D=1024, H=16, Dh=64) on 8 trn2 cores.

Sharding: core c = (batch b = c//4, head-quadrant g = c%4) -> 4 heads of one
batch per core.  bf16 datapath (fp32 PSUM accumulation):
  - Q^T/K^T per head-pair projected from host-pretransposed x^T (bf16),
  - causal flash-style attention in the transposed orientation
    (s_T[j,q] = K^T.T @ Q^T, exp on ScalarE -> bf16, causal mask applied
    post-exp multiplicatively on DVE, P.V + row-sum matmuls in PSUM),
  - software-pipelined emission: PV lags QK by one batch, normalize lags
    its strip by one strip, diagonal (masked) batches run first in each
    strip, pair-1 QK projection and the output projection gap-fill the PE
    between strips,
  - per-strip normalize 1/s = exp(-ln s) on ScalarE (one fused [1,1024]
    row for both heads), partition broadcast on GpSimd, multiply on DVE.
Host sums the 4 partial output projections per batch.
"""

import numpy as np
import ml_dtypes

B, N, D, H, Dh = 2, 2048, 1024, 16, 64
DC = D // 128          # 8 contraction chunks
NB = N // 128          # 16 ctx blocks
NS = N // 512          # 4 q strips
N_CORES = 8
SCALE = float(Dh) ** -0.5

_COMPILED = None
TRACE = False
LAST_EXEC_NS = None
LAST_RESULTS = None


def _build():
    import concourse.bass as bass
    import concourse.tile as tile
    from concourse import bacc, mybir

    f32 = mybir.dt.float32
    bf = mybir.dt.bfloat16
    EXP = mybir.ActivationFunctionType.Exp
    LN = mybir.ActivationFunctionType.Ln

    nc = bacc.Bacc("TRN2", target_bir_lowering=False, debug=False,
                   enable_asserts=False, num_devices=N_CORES)

    # Both Exp and Ln are used (softmax exp + exp(-ln s) normalize).  The
    # table picker binds each function to the first set containing it,
    # which splits them across two sets and inserts a ~1.3us
    # ACT_TABLE_LOAD per switch (17 loads).  Strip Exp/Ln from every other
    # set (in the cached dict, order/ids preserved) so both bind to
    # natural_log_exp_and_others -> a single load.
    import concourse.hw_specs as hw_specs
    tabs = hw_specs.get_activation_tables(nc.m.arch)
    for name, funcs in tabs.items():
        if name != "natural_log_exp_and_others":
            funcs.discard(EXP)
            funcs.discard(LN)

    xT = nc.dram_tensor("xT", [D, N], bf, kind="ExternalInput")
    # weights are host-preswizzled to the SBUF layout [128, c, n] so the
    # DMAs are contiguous 4KB-per-partition (strided 512B-segment loads get
    # starved to ~1/15th bandwidth behind the x^T chunk stream)
    wq = nc.dram_tensor("wq", [128, DC * 256], bf, kind="ExternalInput")
    wk = nc.dram_tensor("wk", [128, DC * 256], bf, kind="ExternalInput")
    wv = nc.dram_tensor("wv", [128, DC * 256], bf, kind="ExternalInput")
    wo = nc.dram_tensor("wo", [128, 2 * D], bf, kind="ExternalInput")
    bo = nc.dram_tensor("bo", [D], f32, kind="ExternalInput")
    kmask = nc.dram_tensor("kmask", [128, 128], bf, kind="ExternalInput")
    y = nc.dram_tensor("y", [N, D], f32, kind="ExternalOutput")

    with tile.TileContext(nc) as tc:
        from contextlib import ExitStack
        with ExitStack() as ctx:
            const = ctx.enter_context(tc.tile_pool(name="const", bufs=1))
            work = ctx.enter_context(tc.tile_pool(name="work", bufs=3))
            epool = ctx.enter_context(tc.tile_pool(name="epool", bufs=5))
            pssT = ctx.enter_context(
                tc.tile_pool(name="pssT", bufs=2, space=bass.MemorySpace.PSUM))
            pspv = ctx.enter_context(
                tc.tile_pool(name="pspv", bufs=2, space=bass.MemorySpace.PSUM))

            # ---------------- loads ----------------
            # ONE serial Sync queue in exact consumption order: parallel
            # queues round-robin packets, which spreads every transfer's
            # completion across the whole ~18us load window; serial FIFO
            # gives wq at ~4us and one x^T chunk every ~1.6us, matching the
            # d-interleaved projection's consumption rate.
            wq_sb = const.tile([128, DC, 256], bf)
            nc.sync.dma_start(wq_sb[:].rearrange("p c n -> p (c n)"), wq.ap())
            wk_sb = const.tile([128, DC, 256], bf)
            nc.sync.dma_start(wk_sb[:].rearrange("p c n -> p (c n)"), wk.ap())

            xs = []
            for d in range(DC):
                xd = const.tile([128, N], bf, name=f"x{d}")
                nc.sync.dma_start(xd[:], xT.ap()[128 * d:128 * d + 128, :])
                xs.append(xd)

            wv_sb = const.tile([128, DC, 256], bf)
            nc.sync.dma_start(wv_sb[:].rearrange("p c n -> p (c n)"), wv.ap())
            wo_sb = const.tile([128, 2, D], bf)
            nc.sync.dma_start(wo_sb[:].rearrange("p c n -> p (c n)"), wo.ap())
            kmask_sb = const.tile([128, 128], bf)
            nc.sync.dma_start(kmask_sb[:], kmask.ap())
            bo_sb = const.tile([1, D], f32)
            nc.sync.dma_start(bo_sb[:], bo.ap().rearrange("(a n) -> a n", a=1))
            Bb = const.tile([128, D], f32)
            nc.gpsimd.partition_broadcast(Bb[:], bo_sb[0:1, :])

            # ---------------- QKV projections ----------------
            qkdst = {}

            def proj_qk_part(p, w_sb, nm, half, interleave=False):
                """One [128,1024] accumulation group of the Q^T or K^T
                projection for head-pair p (emitted piecewise so the groups
                can be spread across the schedule)."""
                key = (nm, p)
                if key not in qkdst:
                    qkdst[key] = const.tile([128, N], bf, tag=f"{nm}T{p}",
                                            name=f"{nm}T{p}")
                dst = qkdst[key]
                hsl = slice(1024 * half, 1024 * half + 1024)
                pq = pssT.tile([128, 1024], f32, tag="sT",
                               name=f"p{nm}{p}{half}")
                for d in range(DC):
                    for ns in (0, 1):
                        osl = slice(512 * ns, 512 * ns + 512)
                        nsl = slice(1024 * half + 512 * ns,
                                    1024 * half + 512 * ns + 512)
                        nc.tensor.matmul(
                            pq[:, osl], w_sb[:, d, 128 * p:128 * p + 128],
                            xs[d][:, nsl],
                            start=(d == 0), stop=(d == DC - 1),
                            skip_group_check=interleave)
                nc.vector.tensor_copy(dst[:, hsl], pq[:])
                return dst

            def proj_qk0():
                # pair 0, Q and K interleaved per d-chunk so the PE consumes
                # the x^T DMA stream as it lands
                qdst = kdst = None
                for half in (0, 1):
                    hsl = slice(1024 * half, 1024 * half + 1024)
                    pq = pssT.tile([128, 1024], f32, tag="sT", name=f"pq0{half}")
                    pk = pssT.tile([128, 1024], f32, tag="sT", name=f"pk0{half}")
                    for d in range(DC):
                        for ns in (0, 1):
                            osl = slice(512 * ns, 512 * ns + 512)
                            nsl = slice(1024 * half + 512 * ns,
                                        1024 * half + 512 * ns + 512)
                            nc.tensor.matmul(
                                pq[:, osl], wq_sb[:, d, 0:128], xs[d][:, nsl],
                                start=(d == 0), stop=(d == DC - 1),
                                skip_group_check=True)
                            nc.tensor.matmul(
                                pk[:, osl], wk_sb[:, d, 0:128], xs[d][:, nsl],
                                start=(d == 0), stop=(d == DC - 1),
                                skip_group_check=True)
                    if qdst is None:
                        qdst = qkdst[("q", 0)] = const.tile(
                            [128, N], bf, tag="qT0", name="qT0")
                        kdst = qkdst[("k", 0)] = const.tile(
                            [128, N], bf, tag="kT0", name="kT0")
                    nc.vector.tensor_copy(qdst[:, hsl], pq[:])
                    nc.vector.tensor_copy(kdst[:, hsl], pk[:])
                return qdst, kdst

            vsb = const.tile([128, NB, 4, Dh + 1], bf)
            nc.vector.memset(vsb[:, :, :, Dh:Dh + 1], 1.0)

            def proj_v():
                for nb in range(NB):
                    pvp = pspv.tile([128, 256], f32, tag="pv", name=f"pvp{nb}")
                    for d in range(DC):
                        nc.tensor.matmul(
                            pvp[:], xs[d][:, 128 * nb:128 * nb + 128],
                            wv_sb[:, d, :], start=(d == 0), stop=(d == DC - 1))
                    nc.vector.tensor_copy(
                        vsb[:, nb, :, 0:Dh],
                        pvp[:].rearrange("p (h d) -> p h d", h=4))

            onorm = [const.tile([128, N], bf, tag="onorm0", name="onorm0"),
                     const.tile([128, N], bf, tag="onorm1", name="onorm1")]

            def out_proj_strip(s):
                for qb in range(4 * s, 4 * s + 4):
                    qsl = slice(128 * qb, 128 * qb + 128)
                    yp = pssT.tile([128, 1024], f32, tag="sT", name=f"yp{qb}")
                    for nst in (0, 1):
                        osl = slice(512 * nst, 512 * nst + 512)
                        for p in (0, 1):
                            nc.tensor.matmul(yp[:, osl], onorm[p][:, qsl],
                                             wo_sb[:, p, osl],
                                             start=(p == 0), stop=(p == 1))
                    ysb = work.tile([128, D], f32, tag="ysb", name=f"ysb{qb}")
                    nc.vector.tensor_add(ysb[:], yp[:], Bb[:])
                    nc.sync.dma_start(y.ap()[qsl, :], ysb[:])

            QT = [None, None]
            KT = [None, None]
            pvs = {}

            def attn_strip(p, s):
                """QK / exp / mask / PV for strip s of pair p.  PV lags QK by
                one batch; diagonal (masked) batches first."""
                qsl0 = 512 * s
                pv = pspv.tile([65, 1024], f32, tag="pv", name=f"pv{p}{s}")
                pvs[(p, s)] = pv
                batches = [[(4 * s, 512, 0), (4 * s + 1, 384, 512)],
                           [(4 * s + 2, 256, 0), (4 * s + 3, 128, 256)]]
                for j0 in range(0, 4 * s, 2):
                    batches.append([(j0, 512, 0), (j0 + 1, 512, 512)])
                first_j = batches[0][0][0]
                last_j = batches[-1][-1][0]

                def emit_qk(batch, bi):
                    # a/b interleaved per entry: adjacent matmuls hit
                    # disjoint row-groups (0-63 / 64-127) and overlap in the
                    # PE array (row tiling)
                    tot = batch[-1][2] + batch[-1][1]
                    sTa = pssT.tile([128, 1024], f32, tag="sT",
                                    name=f"sTa{p}{s}{bi}")
                    sTb = pssT.tile([128, 1024], f32, tag="sT",
                                    name=f"sTb{p}{s}{bi}")
                    for (j, w, ofs) in batch:
                        off = 512 - w
                        jsl = slice(128 * j, 128 * j + 128)
                        qs = slice(qsl0 + off, qsl0 + 512)
                        nc.tensor.matmul(sTa[:, ofs:ofs + w],
                                         KT[p][0:64, jsl], QT[p][0:64, qs],
                                         start=True, stop=True)
                        nc.tensor.matmul(sTb[:, ofs:ofs + w],
                                         KT[p][64:128, jsl], QT[p][64:128, qs],
                                         start=True, stop=True)
                    return sTa, sTb, tot

                def emit_exp_mask(batch, bi, sTa, sTb, tot):
                    ea = epool.tile([128, 1024], bf, tag="e",
                                    name=f"ea{p}{s}{bi}")
                    eb = epool.tile([128, 1024], bf, tag="e",
                                    name=f"eb{p}{s}{bi}")
                    nc.scalar.activation(ea[:, 0:tot], sTa[:, 0:tot], EXP,
                                         scale=SCALE)
                    nc.scalar.activation(eb[:, 0:tot], sTb[:, 0:tot], EXP,
                                         scale=SCALE)
                    for (j, w, ofs) in batch:
                        if j >= 4 * s:  # diagonal chunk: zero masked probs
                            nc.vector.tensor_mul(ea[:, ofs:ofs + 128],
                                                 ea[:, ofs:ofs + 128],
                                                 kmask_sb[:])
                            nc.vector.tensor_mul(eb[:, ofs:ofs + 128],
                                                 eb[:, ofs:ofs + 128],
                                                 kmask_sb[:])
                    return ea, eb

                def emit_pv(batch, ea, eb):
                    # head-a PVs first: they only need ea, so the PE is not
                    # stalled on exp_b (which ScalarE finishes ~1us later)
                    for half, e in ((0, ea), (1, eb)):
                        for (j, w, ofs) in batch:
                            off = 512 - w
                            nc.tensor.matmul(pv[0:65, 512 * half + off:
                                                512 * half + 512],
                                             vsb[:, j, 2 * p + half, :],
                                             e[:, ofs:ofs + w],
                                             start=(j == first_j),
                                             stop=(j == last_j),
                                             skip_group_check=True)

                # PV(b) is emitted after QK(b+1): the PE always has the next
                # batch's QK queued while ScalarE works through exp(b), so
                # the exp round-trip is off the PE critical path.  exp(b) is
                # emitted before QK(b+1) so the sT slot reuse (bufs=2) only
                # ever waits on already-emitted readers.
                pend = (batches[0], 0, *emit_qk(batches[0], 0))
                for bi in range(1, len(batches)):
                    ea, eb = emit_exp_mask(*pend)
                    pbatch = pend[0]
                    pend = (batches[bi], bi, *emit_qk(batches[bi], bi))
                    emit_pv(pbatch, ea, eb)
                ea, eb = emit_exp_mask(*pend)
                emit_pv(pend[0], ea, eb)

            def normalize(p, s):
                """o = pv[0:64] * (1 / pv[64]) per head; one fused row for
                both heads.  Emitted one strip late so every hop's deps are
                long-ready (no FIFO head-of-line blocking on any engine)."""
                pv = pvs.pop((p, s))
                on = onorm[p]
                lnr = work.tile([1, 1024], f32, tag="lnr", name=f"lnr{p}{s}")
                scr = work.tile([1, 1024], f32, tag="scr", name=f"scr{p}{s}")
                nc.scalar.activation(lnr[:], pv[64:65, :], LN)
                nc.scalar.activation(scr[:], lnr[:], EXP, scale=-1.0)
                R = work.tile([64, 1024], f32, tag="R", name=f"R{p}{s}")
                nc.gpsimd.partition_broadcast(R[:], scr[:])
                qs = slice(512 * s, 512 * s + 512)
                nc.vector.tensor_mul(on[0:64, qs], pv[0:64, 0:512],
                                     R[:, 0:512])
                nc.vector.tensor_mul(on[64:128, qs], pv[0:64, 512:1024],
                                     R[:, 512:1024])

            # ---------------- schedule ----------------
            QT[0], KT[0] = proj_qk0()
            proj_v()
            attn_strip(0, 0)
            proj_qk_part(1, wq_sb, "q", 0)
            attn_strip(0, 1)
            normalize(0, 0)
            proj_qk_part(1, wq_sb, "q", 1)
            attn_strip(0, 2)
            normalize(0, 1)
            proj_qk_part(1, wk_sb, "k", 0)
            attn_strip(0, 3)
            normalize(0, 2)
            proj_qk_part(1, wk_sb, "k", 1)
            QT[1] = qkdst[("q", 1)]
            KT[1] = qkdst[("k", 1)]
            # pair-1 strips longest-first: the kernel tail (last strip ->
            # normalize -> out-proj -> store) then hangs off the SHORT strip
            attn_strip(1, 3)
            normalize(0, 3)
            attn_strip(1, 2)
            normalize(1, 3)
            out_proj_strip(3)
            attn_strip(1, 1)
            normalize(1, 2)
            out_proj_strip(2)
            attn_strip(1, 0)
            normalize(1, 1)
            out_proj_strip(1)
            normalize(1, 0)
            out_proj_strip(0)

    nc.compile()
    return nc


def _get_compiled():
    global _COMPILED
    if _COMPILED is None:
        _COMPILED = _build()
    return _COMPILED


def kernel(x, w_qkv, w_out, b_out):
    global LAST_EXEC_NS, LAST_RESULTS
    from concourse.bass_utils import run_bass_kernel_spmd

    bf16 = ml_dtypes.bfloat16
    x = np.asarray(x, dtype=np.float32)
    w_qkv = np.asarray(w_qkv, dtype=np.float32)
    w_out = np.asarray(w_out, dtype=np.float32)
    b_out = np.asarray(b_out, dtype=np.float32)

    kmask_np = np.triu(np.ones((128, 128), dtype=bf16), 0)

    nc = _get_compiled()
    in_maps = []
    for c in range(N_CORES):
        b, g = divmod(c, 4)
        hs = [4 * g + i for i in range(4)]
        cols = np.concatenate([np.arange(64 * h, 64 * h + 64) for h in hs])
        # swizzle [D, n] -> SBUF layout [128, c, n] flattened (contiguous DMA)
        def swz(w):
            return np.ascontiguousarray(
                w.reshape(-1, 128, w.shape[1]).transpose(1, 0, 2)
                .reshape(128, -1).astype(bf16))

        in_maps.append({
            "xT": np.ascontiguousarray(x[b].T.astype(bf16)),
            "wq": swz(w_qkv[:, cols]),
            "wk": swz(w_qkv[:, D + cols]),
            "wv": swz(w_qkv[:, 2 * D + cols]),
            "wo": swz(w_out[cols, :]),
            "bo": b_out if g == 0 else np.zeros_like(b_out),
            "kmask": kmask_np,
        })
    res = run_bass_kernel_spmd(nc, in_maps, core_ids=list(range(N_CORES)),
                               trace=TRACE)
    LAST_EXEC_NS = res.exec_time_ns
    LAST_RESULTS = res
    ys = [res.results[c]["y"] for c in range(N_CORES)]
    out = np.stack([ys[0] + ys[1] + ys[2] + ys[3],
                    ys[4] + ys[5] + ys[6] + ys[7]])
    return out.astype(np.float32)


# revision 26
# speedup vs baseline: 1.1867x; 1.0019x over previous
"""Causal multi-head attention (B=2, N=2048, D=1024, H=16, Dh=64) on 8 trn2 cores.

Sharding: core c = (batch b = c//4, head-quadrant g = c%4) -> 4 heads of one
batch per core.  bf16 datapath (fp32 PSUM accumulation):
  - Q^T/K^T per head-pair projected from host-pretransposed x^T (bf16),
  - causal flash-style attention in the transposed orientation
    (s_T[j,q] = K^T.T @ Q^T, exp on ScalarE -> bf16, causal mask applied
    post-exp multiplicatively on DVE, P.V + row-sum matmuls in PSUM),
  - software-pipelined emission: PV lags QK by one batch, normalize lags
    its strip by one strip, diagonal (masked) batches run first in each
    strip, pair-1 QK projection and the output projection gap-fill the PE
    between strips,
  - per-strip normalize 1/s = exp(-ln s) on ScalarE (one fused [1,1024]
    row for both heads), partition broadcast on GpSimd, multiply on DVE.
Host sums the 4 partial output projections per batch.
"""

import numpy as np
import ml_dtypes

B, N, D, H, Dh = 2, 2048, 1024, 16, 64
DC = D // 128          # 8 contraction chunks
NB = N // 128          # 16 ctx blocks
NS = N // 512          # 4 q strips
N_CORES = 8
SCALE = float(Dh) ** -0.5

_COMPILED = None
TRACE = False
LAST_EXEC_NS = None
LAST_RESULTS = None


def _build():
    import concourse.bass as bass
    import concourse.tile as tile
    from concourse import bacc, mybir

    f32 = mybir.dt.float32
    bf = mybir.dt.bfloat16
    EXP = mybir.ActivationFunctionType.Exp
    LN = mybir.ActivationFunctionType.Ln

    nc = bacc.Bacc("TRN2", target_bir_lowering=False, debug=False,
                   enable_asserts=False, num_devices=N_CORES)

    # Both Exp and Ln are used (softmax exp + exp(-ln s) normalize).  The
    # table picker binds each function to the first set containing it,
    # which splits them across two sets and inserts a ~1.3us
    # ACT_TABLE_LOAD per switch (17 loads).  Strip Exp/Ln from every other
    # set (in the cached dict, order/ids preserved) so both bind to
    # natural_log_exp_and_others -> a single load.
    import concourse.hw_specs as hw_specs
    tabs = hw_specs.get_activation_tables(nc.m.arch)
    for name, funcs in tabs.items():
        if name != "natural_log_exp_and_others":
            funcs.discard(EXP)
            funcs.discard(LN)

    xT = nc.dram_tensor("xT", [D, N], bf, kind="ExternalInput")
    # weights are host-preswizzled to the SBUF layout [128, c, n] so the
    # DMAs are contiguous 4KB-per-partition (strided 512B-segment loads get
    # starved to ~1/15th bandwidth behind the x^T chunk stream)
    wq = nc.dram_tensor("wq", [128, DC * 256], bf, kind="ExternalInput")
    wk = nc.dram_tensor("wk", [128, DC * 256], bf, kind="ExternalInput")
    wv = nc.dram_tensor("wv", [128, DC * 256], bf, kind="ExternalInput")
    wo = nc.dram_tensor("wo", [128, 2 * D], bf, kind="ExternalInput")
    bo = nc.dram_tensor("bo", [D], f32, kind="ExternalInput")
    kmask = nc.dram_tensor("kmask", [128, 128], bf, kind="ExternalInput")
    y = nc.dram_tensor("y", [N, D], f32, kind="ExternalOutput")

    with tile.TileContext(nc) as tc:
        from contextlib import ExitStack
        with ExitStack() as ctx:
            const = ctx.enter_context(tc.tile_pool(name="const", bufs=1))
            work = ctx.enter_context(tc.tile_pool(name="work", bufs=3))
            epool = ctx.enter_context(tc.tile_pool(name="epool", bufs=5))
            pssT = ctx.enter_context(
                tc.tile_pool(name="pssT", bufs=2, space=bass.MemorySpace.PSUM))
            pspv = ctx.enter_context(
                tc.tile_pool(name="pspv", bufs=2, space=bass.MemorySpace.PSUM))

            # ---------------- loads ----------------
            # ONE serial Sync queue in exact consumption order: parallel
            # queues round-robin packets, which spreads every transfer's
            # completion across the whole ~18us load window; serial FIFO
            # gives wq at ~4us and one x^T chunk every ~1.6us, matching the
            # d-interleaved projection's consumption rate.
            wq_sb = const.tile([128, DC, 256], bf)
            nc.sync.dma_start(wq_sb[:].rearrange("p c n -> p (c n)"), wq.ap())
            wk_sb = const.tile([128, DC, 256], bf)
            nc.sync.dma_start(wk_sb[:].rearrange("p c n -> p (c n)"), wk.ap())

            xs = []
            for d in range(DC):
                xd = const.tile([128, N], bf, name=f"x{d}")
                nc.sync.dma_start(xd[:], xT.ap()[128 * d:128 * d + 128, :])
                xs.append(xd)

            wv_sb = const.tile([128, DC, 256], bf)
            nc.sync.dma_start(wv_sb[:].rearrange("p c n -> p (c n)"), wv.ap())
            wo_sb = const.tile([128, 2, D], bf)
            nc.sync.dma_start(wo_sb[:].rearrange("p c n -> p (c n)"), wo.ap())
            kmask_sb = const.tile([128, 128], bf)
            nc.sync.dma_start(kmask_sb[:], kmask.ap())
            bo_sb = const.tile([1, D], f32)
            nc.sync.dma_start(bo_sb[:], bo.ap().rearrange("(a n) -> a n", a=1))
            Bb = const.tile([128, D], f32)
            nc.gpsimd.partition_broadcast(Bb[:], bo_sb[0:1, :])

            # ---------------- QKV projections ----------------
            qkdst = {}

            def proj_qk_part(p, w_sb, nm, half, interleave=False):
                """One [128,1024] accumulation group of the Q^T or K^T
                projection for head-pair p (emitted piecewise so the groups
                can be spread across the schedule)."""
                key = (nm, p)
                if key not in qkdst:
                    qkdst[key] = const.tile([128, N], bf, tag=f"{nm}T{p}",
                                            name=f"{nm}T{p}")
                dst = qkdst[key]
                hsl = slice(1024 * half, 1024 * half + 1024)
                pq = pssT.tile([128, 1024], f32, tag="sT",
                               name=f"p{nm}{p}{half}")
                for d in range(DC):
                    for ns in (0, 1):
                        osl = slice(512 * ns, 512 * ns + 512)
                        nsl = slice(1024 * half + 512 * ns,
                                    1024 * half + 512 * ns + 512)
                        nc.tensor.matmul(
                            pq[:, osl], w_sb[:, d, 128 * p:128 * p + 128],
                            xs[d][:, nsl],
                            start=(d == 0), stop=(d == DC - 1),
                            skip_group_check=interleave)
                nc.vector.tensor_copy(dst[:, hsl], pq[:])
                return dst

            def proj_qk0():
                # pair 0, Q and K interleaved per d-chunk so the PE consumes
                # the x^T DMA stream as it lands
                qdst = kdst = None
                for half in (0, 1):
                    hsl = slice(1024 * half, 1024 * half + 1024)
                    pq = pssT.tile([128, 1024], f32, tag="sT", name=f"pq0{half}")
                    pk = pssT.tile([128, 1024], f32, tag="sT", name=f"pk0{half}")
                    for d in range(DC):
                        for ns in (0, 1):
                            osl = slice(512 * ns, 512 * ns + 512)
                            nsl = slice(1024 * half + 512 * ns,
                                        1024 * half + 512 * ns + 512)
                            nc.tensor.matmul(
                                pq[:, osl], wq_sb[:, d, 0:128], xs[d][:, nsl],
                                start=(d == 0), stop=(d == DC - 1),
                                skip_group_check=True)
                            nc.tensor.matmul(
                                pk[:, osl], wk_sb[:, d, 0:128], xs[d][:, nsl],
                                start=(d == 0), stop=(d == DC - 1),
                                skip_group_check=True)
                    if qdst is None:
                        qdst = qkdst[("q", 0)] = const.tile(
                            [128, N], bf, tag="qT0", name="qT0")
                        kdst = qkdst[("k", 0)] = const.tile(
                            [128, N], bf, tag="kT0", name="kT0")
                    nc.vector.tensor_copy(qdst[:, hsl], pq[:])
                    nc.vector.tensor_copy(kdst[:, hsl], pk[:])
                return qdst, kdst

            vsb = const.tile([128, NB, 4, Dh + 1], bf)
            nc.vector.memset(vsb[:, :, :, Dh:Dh + 1], 1.0)

            def proj_v():
                for nb in range(NB):
                    pvp = pspv.tile([128, 256], f32, tag="pv", name=f"pvp{nb}")
                    for d in range(DC):
                        nc.tensor.matmul(
                            pvp[:], xs[d][:, 128 * nb:128 * nb + 128],
                            wv_sb[:, d, :], start=(d == 0), stop=(d == DC - 1))
                    nc.vector.tensor_copy(
                        vsb[:, nb, :, 0:Dh],
                        pvp[:].rearrange("p (h d) -> p h d", h=4))

            onorm = [const.tile([128, N], bf, tag="onorm0", name="onorm0"),
                     const.tile([128, N], bf, tag="onorm1", name="onorm1")]

            def out_proj_strip(s):
                for qb in range(4 * s, 4 * s + 4):
                    qsl = slice(128 * qb, 128 * qb + 128)
                    yp = pssT.tile([128, 1024], f32, tag="sT", name=f"yp{qb}")
                    for nst in (0, 1):
                        osl = slice(512 * nst, 512 * nst + 512)
                        for p in (0, 1):
                            nc.tensor.matmul(yp[:, osl], onorm[p][:, qsl],
                                             wo_sb[:, p, osl],
                                             start=(p == 0), stop=(p == 1))
                    ysb = work.tile([128, D], f32, tag="ysb", name=f"ysb{qb}")
                    nc.vector.tensor_add(ysb[:], yp[:], Bb[:])
                    nc.sync.dma_start(y.ap()[qsl, :], ysb[:])

            QT = [None, None]
            KT = [None, None]
            pvs = {}

            def attn_strip(p, s):
                """QK / exp / mask / PV for strip s of pair p.  PV lags QK by
                one batch; diagonal (masked) batches first."""
                qsl0 = 512 * s
                pv = pspv.tile([65, 1024], f32, tag="pv", name=f"pv{p}{s}")
                pvs[(p, s)] = pv
                batches = [[(4 * s, 512, 0), (4 * s + 1, 384, 512)],
                           [(4 * s + 2, 256, 0), (4 * s + 3, 128, 256)]]
                for j0 in range(0, 4 * s, 2):
                    batches.append([(j0, 512, 0), (j0 + 1, 512, 512)])
                first_j = batches[0][0][0]
                last_j = batches[-1][-1][0]

                def emit_qk(batch, bi):
                    # a/b interleaved per entry: adjacent matmuls hit
                    # disjoint row-groups (0-63 / 64-127) and overlap in the
                    # PE array (row tiling)
                    tot = batch[-1][2] + batch[-1][1]
                    sTa = pssT.tile([128, 1024], f32, tag="sT",
                                    name=f"sTa{p}{s}{bi}")
                    sTb = pssT.tile([128, 1024], f32, tag="sT",
                                    name=f"sTb{p}{s}{bi}")
                    for (j, w, ofs) in batch:
                        off = 512 - w
                        jsl = slice(128 * j, 128 * j + 128)
                        qs = slice(qsl0 + off, qsl0 + 512)
                        nc.tensor.matmul(sTa[:, ofs:ofs + w],
                                         KT[p][0:64, jsl], QT[p][0:64, qs],
                                         start=True, stop=True)
                        nc.tensor.matmul(sTb[:, ofs:ofs + w],
                                         KT[p][64:128, jsl], QT[p][64:128, qs],
                                         start=True, stop=True)
                    return sTa, sTb, tot

                def emit_exp_mask(batch, bi, sTa, sTb, tot):
                    ea = epool.tile([128, 1024], bf, tag="e",
                                    name=f"ea{p}{s}{bi}")
                    eb = epool.tile([128, 1024], bf, tag="e",
                                    name=f"eb{p}{s}{bi}")
                    nc.scalar.activation(ea[:, 0:tot], sTa[:, 0:tot], EXP,
                                         scale=SCALE)
                    nc.scalar.activation(eb[:, 0:tot], sTb[:, 0:tot], EXP,
                                         scale=SCALE)
                    for (j, w, ofs) in batch:
                        if j >= 4 * s:  # diagonal chunk: zero masked probs
                            nc.vector.tensor_mul(ea[:, ofs:ofs + 128],
                                                 ea[:, ofs:ofs + 128],
                                                 kmask_sb[:])
                            nc.vector.tensor_mul(eb[:, ofs:ofs + 128],
                                                 eb[:, ofs:ofs + 128],
                                                 kmask_sb[:])
                    return ea, eb

                def emit_pv(batch, ea, eb):
                    # head-a PVs first: they only need ea, so the PE is not
                    # stalled on exp_b (which ScalarE finishes ~1us later)
                    for half, e in ((0, ea), (1, eb)):
                        for (j, w, ofs) in batch:
                            off = 512 - w
                            nc.tensor.matmul(pv[0:65, 512 * half + off:
                                                512 * half + 512],
                                             vsb[:, j, 2 * p + half, :],
                                             e[:, ofs:ofs + w],
                                             start=(j == first_j),
                                             stop=(j == last_j),
                                             skip_group_check=True)

                # PV(b) is emitted after QK(b+1): the PE always has the next
                # batch's QK queued while ScalarE works through exp(b), so
                # the exp round-trip is off the PE critical path.  exp(b) is
                # emitted before QK(b+1) so the sT slot reuse (bufs=2) only
                # ever waits on already-emitted readers.
                pend = (batches[0], 0, *emit_qk(batches[0], 0))
                for bi in range(1, len(batches)):
                    ea, eb = emit_exp_mask(*pend)
                    pbatch = pend[0]
                    pend = (batches[bi], bi, *emit_qk(batches[bi], bi))
                    emit_pv(pbatch, ea, eb)
                ea, eb = emit_exp_mask(*pend)
                emit_pv(pend[0], ea, eb)

            def normalize(p, s):
                """o = pv[0:64] * (1 / pv[64]) per head; one fused row for
                both heads.  Emitted one strip late so every hop's deps are
                long-ready (no FIFO head-of-line blocking on any engine)."""
                pv = pvs.pop((p, s))
                on = onorm[p]
                lnr = work.tile([1, 1024], f32, tag="lnr", name=f"lnr{p}{s}")
                scr = work.tile([1, 1024], f32, tag="scr", name=f"scr{p}{s}")
                nc.scalar.activation(lnr[:], pv[64:65, :], LN)
                nc.scalar.activation(scr[:], lnr[:], EXP, scale=-1.0)
                R = work.tile([64, 1024], f32, tag="R", name=f"R{p}{s}")
                nc.gpsimd.partition_broadcast(R[:], scr[:])
                qs = slice(512 * s, 512 * s + 512)
                nc.vector.tensor_mul(on[0:64, qs], pv[0:64, 0:512],
                                     R[:, 0:512])
                nc.vector.tensor_mul(on[64:128, qs], pv[0:64, 512:1024],
                                     R[:, 512:1024])

            # ---------------- schedule ----------------
            QT[0], KT[0] = proj_qk0()
            proj_v()
            attn_strip(0, 0)
            proj_qk_part(1, wq_sb, "q", 0)
            attn_strip(0, 1)
            normalize(0, 0)
            proj_qk_part(1, wq_sb, "q", 1)
            attn_strip(0, 2)
            normalize(0, 1)
            proj_qk_part(1, wk_sb, "k", 0)
            attn_strip(0, 3)
            normalize(0, 2)
            proj_qk_part(1, wk_sb, "k", 1)
            QT[1] = qkdst[("q", 1)]
            KT[1] = qkdst[("k", 1)]
            attn_strip(1, 0)
            normalize(0, 3)
            attn_strip(1, 1)
            normalize(1, 0)
            out_proj_strip(0)
            attn_strip(1, 2)
            normalize(1, 1)
            out_proj_strip(1)
            attn_strip(1, 3)
            normalize(1, 2)
            out_proj_strip(2)
            normalize(1, 3)
            out_proj_strip(3)

    nc.compile()
    return nc


def _get_compiled():
    global _COMPILED
    if _COMPILED is None:
        _COMPILED = _build()
    return _COMPILED


def kernel(x, w_qkv, w_out, b_out):
    global LAST_EXEC_NS, LAST_RESULTS
    from concourse.bass_utils import run_bass_kernel_spmd

    bf16 = ml_dtypes.bfloat16
    x = np.asarray(x, dtype=np.float32)
    w_qkv = np.asarray(w_qkv, dtype=np.float32)
    w_out = np.asarray(w_out, dtype=np.float32)
    b_out = np.asarray(b_out, dtype=np.float32)

    kmask_np = np.triu(np.ones((128, 128), dtype=bf16), 0)

    nc = _get_compiled()
    in_maps = []
    for c in range(N_CORES):
        b, g = divmod(c, 4)
        hs = [4 * g + i for i in range(4)]
        cols = np.concatenate([np.arange(64 * h, 64 * h + 64) for h in hs])
        # swizzle [D, n] -> SBUF layout [128, c, n] flattened (contiguous DMA)
        def swz(w):
            return np.ascontiguousarray(
                w.reshape(-1, 128, w.shape[1]).transpose(1, 0, 2)
                .reshape(128, -1).astype(bf16))

        in_maps.append({
            "xT": np.ascontiguousarray(x[b].T.astype(bf16)),
            "wq": swz(w_qkv[:, cols]),
            "wk": swz(w_qkv[:, D + cols]),
            "wv": swz(w_qkv[:, 2 * D + cols]),
            "wo": swz(w_out[cols, :]),
            "bo": b_out if g == 0 else np.zeros_like(b_out),
            "kmask": kmask_np,
        })
    res = run_bass_kernel_spmd(nc, in_maps, core_ids=list(range(N_CORES)),
                               trace=TRACE)
    LAST_EXEC_NS = res.exec_time_ns
    LAST_RESULTS = res
    ys = [res.results[c]["y"] for c in range(N_CORES)]
    out = np.stack([ys[0] + ys[1] + ys[2] + ys[3],
                    ys[4] + ys[5] + ys[6] + ys[7]])
    return out.astype(np.float32)


# revision 28
# speedup vs baseline: 1.3843x; 1.1665x over previous
"""Causal multi-head attention (B=2, N=2048, D=1024, H=16, Dh=64) on 8 trn2 cores.

Sharding: core c = (batch b = c//4, head-quadrant g = c%4) -> 4 heads of one
batch per core.  bf16 datapath (fp32 PSUM accumulation):
  - Q^T/K^T per head-pair projected from host-pretransposed x^T (bf16),
  - causal flash-style attention in the transposed orientation
    (s_T[j,q] = K^T.T @ Q^T, exp on ScalarE -> bf16, causal mask applied
    post-exp multiplicatively on DVE, P.V + row-sum matmuls in PSUM),
  - software-pipelined emission: PV lags QK by one batch, normalize lags
    its strip by one strip, diagonal (masked) batches run first in each
    strip, pair-1 QK projection and the output projection gap-fill the PE
    between strips,
  - per-strip normalize 1/s = exp(-ln s) on ScalarE (one fused [1,1024]
    row for both heads), partition broadcast on GpSimd, multiply on DVE.
Host sums the 4 partial output projections per batch.
"""

import numpy as np
import ml_dtypes

B, N, D, H, Dh = 2, 2048, 1024, 16, 64
DC = D // 128          # 8 contraction chunks
NB = N // 128          # 16 ctx blocks
NS = N // 512          # 4 q strips
N_CORES = 8
SCALE = float(Dh) ** -0.5

_COMPILED = None
TRACE = False
LAST_EXEC_NS = None
LAST_RESULTS = None


def _build():
    import concourse.bass as bass
    import concourse.tile as tile
    from concourse import bacc, mybir

    f32 = mybir.dt.float32
    bf = mybir.dt.bfloat16
    EXP = mybir.ActivationFunctionType.Exp
    LN = mybir.ActivationFunctionType.Ln

    nc = bacc.Bacc("TRN2", target_bir_lowering=False, debug=False,
                   enable_asserts=False, num_devices=N_CORES)

    # Both Exp and Ln are used (softmax exp + exp(-ln s) normalize).  The
    # table picker binds each function to the first set containing it,
    # which splits them across two sets and inserts a ~1.3us
    # ACT_TABLE_LOAD per switch (17 loads).  Strip Exp/Ln from every other
    # set (in the cached dict, order/ids preserved) so both bind to
    # natural_log_exp_and_others -> a single load.
    import concourse.hw_specs as hw_specs
    tabs = hw_specs.get_activation_tables(nc.m.arch)
    for name, funcs in tabs.items():
        if name != "natural_log_exp_and_others":
            funcs.discard(EXP)
            funcs.discard(LN)

    xT = nc.dram_tensor("xT", [D, N], bf, kind="ExternalInput")
    # weights are host-preswizzled to the SBUF layout [128, c, n] so the
    # DMAs are contiguous 4KB-per-partition (strided 512B-segment loads get
    # starved to ~1/15th bandwidth behind the x^T chunk stream)
    wq = nc.dram_tensor("wq", [128, DC * 256], bf, kind="ExternalInput")
    wk = nc.dram_tensor("wk", [128, DC * 256], bf, kind="ExternalInput")
    wv = nc.dram_tensor("wv", [128, DC * 256], bf, kind="ExternalInput")
    wo = nc.dram_tensor("wo", [128, 2 * D], bf, kind="ExternalInput")
    bo = nc.dram_tensor("bo", [D], f32, kind="ExternalInput")
    kmask = nc.dram_tensor("kmask", [128, 128], bf, kind="ExternalInput")
    y = nc.dram_tensor("y", [N, D], f32, kind="ExternalOutput")

    with tile.TileContext(nc) as tc:
        from contextlib import ExitStack
        with ExitStack() as ctx:
            const = ctx.enter_context(tc.tile_pool(name="const", bufs=1))
            work = ctx.enter_context(tc.tile_pool(name="work", bufs=3))
            epool = ctx.enter_context(tc.tile_pool(name="epool", bufs=5))
            pssT = ctx.enter_context(
                tc.tile_pool(name="pssT", bufs=2, space=bass.MemorySpace.PSUM))
            pspv = ctx.enter_context(
                tc.tile_pool(name="pspv", bufs=2, space=bass.MemorySpace.PSUM))

            # ---------------- loads ----------------
            # ONE serial Sync queue in exact consumption order: parallel
            # queues round-robin packets, which spreads every transfer's
            # completion across the whole ~18us load window; serial FIFO
            # gives wq at ~4us and one x^T chunk every ~1.6us, matching the
            # d-interleaved projection's consumption rate.
            wq_sb = const.tile([128, DC, 256], bf)
            nc.sync.dma_start(wq_sb[:].rearrange("p c n -> p (c n)"), wq.ap())
            wk_sb = const.tile([128, DC, 256], bf)
            nc.sync.dma_start(wk_sb[:].rearrange("p c n -> p (c n)"), wk.ap())

            xs = []
            for d in range(DC):
                xd = const.tile([128, N], bf, name=f"x{d}")
                nc.sync.dma_start(xd[:], xT.ap()[128 * d:128 * d + 128, :])
                xs.append(xd)

            wv_sb = const.tile([128, DC, 256], bf)
            nc.sync.dma_start(wv_sb[:].rearrange("p c n -> p (c n)"), wv.ap())
            wo_sb = const.tile([128, 2, D], bf)
            nc.sync.dma_start(wo_sb[:].rearrange("p c n -> p (c n)"), wo.ap())
            kmask_sb = const.tile([128, 128], bf)
            nc.sync.dma_start(kmask_sb[:], kmask.ap())
            bo_sb = const.tile([1, D], f32)
            nc.sync.dma_start(bo_sb[:], bo.ap().rearrange("(a n) -> a n", a=1))
            Bb = const.tile([128, D], f32)
            nc.gpsimd.partition_broadcast(Bb[:], bo_sb[0:1, :])

            # ---------------- QKV projections ----------------
            qkdst = {}

            def proj_qk_part(p, w_sb, nm, half, interleave=False):
                """One [128,1024] accumulation group of the Q^T or K^T
                projection for head-pair p (emitted piecewise so the groups
                can be spread across the schedule)."""
                key = (nm, p)
                if key not in qkdst:
                    qkdst[key] = const.tile([128, N], bf, tag=f"{nm}T{p}",
                                            name=f"{nm}T{p}")
                dst = qkdst[key]
                hsl = slice(1024 * half, 1024 * half + 1024)
                pq = pssT.tile([128, 1024], f32, tag="sT",
                               name=f"p{nm}{p}{half}")
                for d in range(DC):
                    for ns in (0, 1):
                        osl = slice(512 * ns, 512 * ns + 512)
                        nsl = slice(1024 * half + 512 * ns,
                                    1024 * half + 512 * ns + 512)
                        nc.tensor.matmul(
                            pq[:, osl], w_sb[:, d, 128 * p:128 * p + 128],
                            xs[d][:, nsl],
                            start=(d == 0), stop=(d == DC - 1),
                            skip_group_check=interleave)
                nc.vector.tensor_copy(dst[:, hsl], pq[:])
                return dst

            def proj_qk0():
                # pair 0, Q and K interleaved per d-chunk so the PE consumes
                # the x^T DMA stream as it lands
                qdst = kdst = None
                for half in (0, 1):
                    hsl = slice(1024 * half, 1024 * half + 1024)
                    pq = pssT.tile([128, 1024], f32, tag="sT", name=f"pq0{half}")
                    pk = pssT.tile([128, 1024], f32, tag="sT", name=f"pk0{half}")
                    for d in range(DC):
                        for ns in (0, 1):
                            osl = slice(512 * ns, 512 * ns + 512)
                            nsl = slice(1024 * half + 512 * ns,
                                        1024 * half + 512 * ns + 512)
                            nc.tensor.matmul(
                                pq[:, osl], wq_sb[:, d, 0:128], xs[d][:, nsl],
                                start=(d == 0), stop=(d == DC - 1),
                                skip_group_check=True)
                            nc.tensor.matmul(
                                pk[:, osl], wk_sb[:, d, 0:128], xs[d][:, nsl],
                                start=(d == 0), stop=(d == DC - 1),
                                skip_group_check=True)
                    if qdst is None:
                        qdst = qkdst[("q", 0)] = const.tile(
                            [128, N], bf, tag="qT0", name="qT0")
                        kdst = qkdst[("k", 0)] = const.tile(
                            [128, N], bf, tag="kT0", name="kT0")
                    nc.vector.tensor_copy(qdst[:, hsl], pq[:])
                    nc.vector.tensor_copy(kdst[:, hsl], pk[:])
                return qdst, kdst

            vsb = const.tile([128, NB, 4, Dh + 1], bf)
            nc.vector.memset(vsb[:, :, :, Dh:Dh + 1], 1.0)

            def proj_v():
                for nb in range(NB):
                    pvp = pspv.tile([128, 256], f32, tag="pv", name=f"pvp{nb}")
                    for d in range(DC):
                        nc.tensor.matmul(
                            pvp[:], xs[d][:, 128 * nb:128 * nb + 128],
                            wv_sb[:, d, :], start=(d == 0), stop=(d == DC - 1))
                    nc.vector.tensor_copy(
                        vsb[:, nb, :, 0:Dh],
                        pvp[:].rearrange("p (h d) -> p h d", h=4))

            onorm = [const.tile([128, N], bf, tag="onorm0", name="onorm0"),
                     const.tile([128, N], bf, tag="onorm1", name="onorm1")]

            def out_proj_strip(s):
                for qb2 in range(2 * s, 2 * s + 2):
                    ysb = work.tile([128, 2, D], f32, tag="ysb", name=f"ysb{qb2}")
                    for i in (0, 1):
                        qb = 2 * qb2 + i
                        qsl = slice(128 * qb, 128 * qb + 128)
                        yp = pssT.tile([128, 1024], f32, tag="sT", name=f"yp{qb}")
                        for nst in (0, 1):
                            osl = slice(512 * nst, 512 * nst + 512)
                            for p in (0, 1):
                                nc.tensor.matmul(yp[:, osl], onorm[p][:, qsl],
                                                 wo_sb[:, p, osl],
                                                 start=(p == 0), stop=(p == 1))
                        nc.vector.tensor_add(ysb[:, i, :], yp[:], Bb[:])
                    nc.sync.dma_start(
                        y.ap()[256 * qb2:256 * qb2 + 256, :]
                        .rearrange("(i p) n -> p i n", p=128),
                        ysb[:])

            QT = [None, None]
            KT = [None, None]
            pvs = {}

            def attn_strip(p, s):
                """QK / exp / mask / PV for strip s of pair p.  PV lags QK by
                one batch; diagonal (masked) batches first."""
                qsl0 = 512 * s
                pv = pspv.tile([65, 1024], f32, tag="pv", name=f"pv{p}{s}")
                pvs[(p, s)] = pv
                batches = [[(4 * s, 512, 0), (4 * s + 1, 384, 512)],
                           [(4 * s + 2, 256, 0), (4 * s + 3, 128, 256)]]
                for j0 in range(0, 4 * s, 2):
                    batches.append([(j0, 512, 0), (j0 + 1, 512, 512)])
                first_j = batches[0][0][0]
                last_j = batches[-1][-1][0]

                def emit_qk(batch, bi):
                    # a/b interleaved per entry: adjacent matmuls hit
                    # disjoint row-groups (0-63 / 64-127) and overlap in the
                    # PE array (row tiling)
                    tot = batch[-1][2] + batch[-1][1]
                    sTa = pssT.tile([128, 1024], f32, tag="sT",
                                    name=f"sTa{p}{s}{bi}")
                    sTb = pssT.tile([128, 1024], f32, tag="sT",
                                    name=f"sTb{p}{s}{bi}")
                    for (j, w, ofs) in batch:
                        off = 512 - w
                        jsl = slice(128 * j, 128 * j + 128)
                        qs = slice(qsl0 + off, qsl0 + 512)
                        nc.tensor.matmul(sTa[:, ofs:ofs + w],
                                         KT[p][0:64, jsl], QT[p][0:64, qs],
                                         start=True, stop=True)
                        nc.tensor.matmul(sTb[:, ofs:ofs + w],
                                         KT[p][64:128, jsl], QT[p][64:128, qs],
                                         start=True, stop=True)
                    return sTa, sTb, tot

                def emit_exp_mask(batch, bi, sTa, sTb, tot):
                    ea = epool.tile([128, 1024], bf, tag="e",
                                    name=f"ea{p}{s}{bi}")
                    eb = epool.tile([128, 1024], bf, tag="e",
                                    name=f"eb{p}{s}{bi}")
                    nc.scalar.activation(ea[:, 0:tot], sTa[:, 0:tot], EXP,
                                         scale=SCALE)
                    nc.scalar.activation(eb[:, 0:tot], sTb[:, 0:tot], EXP,
                                         scale=SCALE)
                    for (j, w, ofs) in batch:
                        if j >= 4 * s:  # diagonal chunk: zero masked probs
                            nc.vector.tensor_mul(ea[:, ofs:ofs + 128],
                                                 ea[:, ofs:ofs + 128],
                                                 kmask_sb[:])
                            nc.vector.tensor_mul(eb[:, ofs:ofs + 128],
                                                 eb[:, ofs:ofs + 128],
                                                 kmask_sb[:])
                    return ea, eb

                def emit_pv(batch, ea, eb):
                    # a/b alternated per entry: consecutive matmuls then
                    # accumulate into different PSUM banks and pipeline
                    for (j, w, ofs) in batch:
                        off = 512 - w
                        first = (j == first_j)
                        last = (j == last_j)
                        nc.tensor.matmul(pv[0:65, off:512],
                                         vsb[:, j, 2 * p + 0, :],
                                         ea[:, ofs:ofs + w],
                                         start=first, stop=last,
                                         skip_group_check=True)
                        nc.tensor.matmul(pv[0:65, 512 + off:1024],
                                         vsb[:, j, 2 * p + 1, :],
                                         eb[:, ofs:ofs + w],
                                         start=first, stop=last,
                                         skip_group_check=True)

                # PV(b) is emitted after QK(b+1): the PE always has the next
                # batch's QK queued while ScalarE works through exp(b), so
                # the exp round-trip is off the PE critical path.  exp(b) is
                # emitted before QK(b+1) so the sT slot reuse (bufs=2) only
                # ever waits on already-emitted readers.
                pend = (batches[0], 0, *emit_qk(batches[0], 0))
                for bi in range(1, len(batches)):
                    ea, eb = emit_exp_mask(*pend)
                    pbatch = pend[0]
                    pend = (batches[bi], bi, *emit_qk(batches[bi], bi))
                    emit_pv(pbatch, ea, eb)
                ea, eb = emit_exp_mask(*pend)
                emit_pv(pend[0], ea, eb)

            def normalize(p, s):
                """o = pv[0:64] * (1 / pv[64]) per head; one fused row for
                both heads.  Emitted one strip late so every hop's deps are
                long-ready (no FIFO head-of-line blocking on any engine)."""
                pv = pvs.pop((p, s))
                on = onorm[p]
                lnr = work.tile([1, 1024], f32, tag="lnr", name=f"lnr{p}{s}")
                scr = work.tile([1, 1024], f32, tag="scr", name=f"scr{p}{s}")
                nc.scalar.activation(lnr[:], pv[64:65, :], LN)
                nc.scalar.activation(scr[:], lnr[:], EXP, scale=-1.0)
                R = work.tile([64, 1024], f32, tag="R", name=f"R{p}{s}")
                nc.gpsimd.partition_broadcast(R[:], scr[:])
                qs = slice(512 * s, 512 * s + 512)
                nc.vector.tensor_mul(on[0:64, qs], pv[0:64, 0:512],
                                     R[:, 0:512])
                nc.vector.tensor_mul(on[64:128, qs], pv[0:64, 512:1024],
                                     R[:, 512:1024])

            # ---------------- schedule ----------------
            QT[0], KT[0] = proj_qk0()
            proj_v()
            attn_strip(0, 0)
            proj_qk_part(1, wq_sb, "q", 0)
            attn_strip(0, 1)
            normalize(0, 0)
            proj_qk_part(1, wq_sb, "q", 1)
            attn_strip(0, 2)
            normalize(0, 1)
            proj_qk_part(1, wk_sb, "k", 0)
            attn_strip(0, 3)
            normalize(0, 2)
            proj_qk_part(1, wk_sb, "k", 1)
            QT[1] = qkdst[("q", 1)]
            KT[1] = qkdst[("k", 1)]
            attn_strip(1, 0)
            normalize(0, 3)
            attn_strip(1, 1)
            normalize(1, 0)
            out_proj_strip(0)
            attn_strip(1, 2)
            normalize(1, 1)
            out_proj_strip(1)
            attn_strip(1, 3)
            normalize(1, 2)
            out_proj_strip(2)
            normalize(1, 3)
            out_proj_strip(3)

    nc.compile()
    return nc


def _get_compiled():
    global _COMPILED
    if _COMPILED is None:
        _COMPILED = _build()
    return _COMPILED


def kernel(x, w_qkv, w_out, b_out):
    global LAST_EXEC_NS, LAST_RESULTS
    from concourse.bass_utils import run_bass_kernel_spmd

    bf16 = ml_dtypes.bfloat16
    x = np.asarray(x, dtype=np.float32)
    w_qkv = np.asarray(w_qkv, dtype=np.float32)
    w_out = np.asarray(w_out, dtype=np.float32)
    b_out = np.asarray(b_out, dtype=np.float32)

    kmask_np = np.triu(np.ones((128, 128), dtype=bf16), 0)

    nc = _get_compiled()
    in_maps = []
    for c in range(N_CORES):
        b, g = divmod(c, 4)
        hs = [4 * g + i for i in range(4)]
        cols = np.concatenate([np.arange(64 * h, 64 * h + 64) for h in hs])
        # swizzle [D, n] -> SBUF layout [128, c, n] flattened (contiguous DMA)
        def swz(w):
            return np.ascontiguousarray(
                w.reshape(-1, 128, w.shape[1]).transpose(1, 0, 2)
                .reshape(128, -1).astype(bf16))

        in_maps.append({
            "xT": np.ascontiguousarray(x[b].T.astype(bf16)),
            "wq": swz(w_qkv[:, cols]),
            "wk": swz(w_qkv[:, D + cols]),
            "wv": swz(w_qkv[:, 2 * D + cols]),
            "wo": swz(w_out[cols, :]),
            "bo": b_out if g == 0 else np.zeros_like(b_out),
            "kmask": kmask_np,
        })
    res = run_bass_kernel_spmd(nc, in_maps, core_ids=list(range(N_CORES)),
                               trace=TRACE)
    LAST_EXEC_NS = res.exec_time_ns
    LAST_RESULTS = res
    ys = [res.results[c]["y"] for c in range(N_CORES)]
    out = np.stack([ys[0] + ys[1] + ys[2] + ys[3],
                    ys[4] + ys[5] + ys[6] + ys[7]])
    return out.astype(np.float32)
